# revision 47
# baseline (speedup 1.0000x reference)
"""Trainium2 Bass kernel for nn_AttentionFusion (B=8192, M=4, H=1024), 8-core data parallel.

Math (exact reformulation of the reference):
  logits[b,m,n] = conf[b,m] * (y_{4b+m} . x_{4b+n}) + conf[b,m]*beta[4b+n]
      (alpha/d rank-1 terms dropped: softmax over n is shift-invariant)
      with Y = X G, G = (Wq/sqrt(H))^T Wk, beta = X (Wk^T bq/32)
  wt[b,n] = sum_m softmax_n(logits)[b,m,n]
  Z[b]    = sum_n wt[b,n] X[4b+n]
  out[b]  = Z[b] (Wo Wv / 4)^T  (+ bias, added on host)

v5: score path (Y = X G, block-diag gram S = Y X^T) in fp8 e4m3 with DoubleRow
double-pumping. The within-batch diagonal extraction is done ON-CHIP (no DRAM
bounce): the gram psum->sbuf copy is a masked STT ((S * conf) * blockdiag mask),
a strided reduce compacts each row's 4 surviving entries, softmax runs in
token-partition layout, and the modality sum is 4 tiny PE matmuls against a 0/1
selector that land the combine weights directly in batch-partition layout.
Value path in bf16 (STT convex combine, XBAR-DMA transpose of Z, bf16 output
projection paired across super-tiles; for the final pair the combine is split
by column halves with quarter transposes so the last projection starts early).
All DRAM tensors are host-prearranged so every streaming DMA is one contiguous
burst per partition.
"""
import sys

if '/opt/trn_rl_repo' not in sys.path:
    sys.path.insert(0, '/opt/trn_rl_repo')

import numpy as np
import ml_dtypes

B, M, H = 8192, 4, 1024
NCORES = 8
B_CORE = B // NCORES            # 1024 batch rows per core
T_CORE = B_CORE * M             # 4096 tokens per core
T_SUPER = 512                   # tokens per super-tile (128 batch rows)
P = 128
OC = H // P                     # 8 output chunks
HC = H // P                     # 8 contraction chunks
F8 = ml_dtypes.float8_e4m3      # TRN e4m3: max normal 240
F16 = ml_dtypes.bfloat16

_NC_CACHE = {}


def build_bass(n_super=T_CORE // T_SUPER):
    import concourse.bass as bass
    import concourse.mybir as mybir
    import concourse.tile as tile
    from concourse import bacc

    assert n_super % 2 == 0
    n_pair = n_super // 2
    t_core = n_super * T_SUPER
    b_core = t_core // M
    b_super = T_SUPER // M                 # 128 batch rows per super-tile

    nc = bacc.Bacc(None, target_bir_lowering=False)
    # all host-prearranged to partition-major contiguous layouts
    xT4 = nc.dram_tensor("xT4", [P, n_super, HC, T_SUPER], mybir.dt.float8e4,
                         kind="ExternalInput")
    xg = nc.dram_tensor("xg", [b_core, M * H], mybir.dt.bfloat16, kind="ExternalInput")
    wg2 = nc.dram_tensor("wg2", [P, OC, HC, P], mybir.dt.float8e4,
                         kind="ExternalInput")
    wc2 = nc.dram_tensor("wc2", [P, OC, HC, P], mybir.dt.bfloat16,
                         kind="ExternalInput")
    # conf in token-partition layout: p = 4*j' + m, per 32-batch-row block tt
    ctok2 = nc.dram_tensor("ctok2", [P, n_super, 4], mybir.dt.float32,
                           kind="ExternalInput")
    # conf*beta in token-partition layout, cols (tt, n)
    cbt2 = nc.dram_tensor("cbt2", [P, n_super, 16], mybir.dt.float32,
                          kind="ExternalInput")
    # blockdiag mask: mask[p, k] = (k//4 == p//4)
    dmask = nc.dram_tensor("dmask", [P, P], mybir.dt.float32,
                           kind="ExternalInput")
    # modality-sum selector: e4[pi, po] = (pi//4 == po)
    e4m = nc.dram_tensor("e4m", [P, P // 4], mybir.dt.bfloat16,
                         kind="ExternalInput")
    syv = nc.dram_tensor("syv", [1], mybir.dt.float32, kind="ExternalInput")
    outT3 = nc.dram_tensor("outT3", [P, n_pair, OC, 2 * b_super], mybir.dt.bfloat16,
                           kind="ExternalOutput")

    FDT = mybir.dt.float32
    DT16 = mybir.dt.bfloat16
    DT8 = mybir.dt.float8e4
    BDT = mybir.dt.bfloat16
    AX = mybir.AxisListType.X
    MUL = mybir.AluOpType.mult
    ADD = mybir.AluOpType.add
    DR = mybir.MatmulPerfMode.DoubleRow
    COPY = mybir.ActivationFunctionType.Copy
    EXP = mybir.ActivationFunctionType.Exp

    from contextlib import ExitStack
    with tile.TileContext(nc) as tc:
        with ExitStack() as _es:
            wp = _es.enter_context(tc.tile_pool(name="wp", bufs=1))
            cp = _es.enter_context(tc.tile_pool(name="cp", bufs=1))
            xp = _es.enter_context(tc.tile_pool(name="xp", bufs=3))
            xgp = _es.enter_context(tc.tile_pool(name="xgp", bufs=2))
            yp = _es.enter_context(tc.tile_pool(name="yp", bufs=3))
            gpl = _es.enter_context(tc.tile_pool(name="gp", bufs=3))
            smp = _es.enter_context(tc.tile_pool(name="smp", bufs=3))
            zp = _es.enter_context(tc.tile_pool(name="zp", bufs=3))
            ztp = _es.enter_context(tc.tile_pool(name="ztp", bufs=3))
            osb = _es.enter_context(tc.tile_pool(name="osb", bufs=3))
            psp = _es.enter_context(tc.tile_pool(name="psp", bufs=3, space="PSUM"))
            psg = _es.enter_context(tc.tile_pool(name="psg", bufs=2, space="PSUM"))
            pso = _es.enter_context(tc.tile_pool(name="pso", bufs=2, space="PSUM"))
            psw = _es.enter_context(tc.tile_pool(name="psw", bufs=1, space="PSUM"))

            # ---- HAM warmup: dataless matmuls spin the PE to the warm clock
            #      while the lead-in DMAs stream ----
            wu = wp.tile([P, P], BDT, tag="warm", name="warm_sb")
            nc.vector.memset(wu[:], 1.0)
            wups = psg.tile([P, P], FDT, tag="gram_ps", name="warm_ps")
            NWARM = 20
            for i in range(NWARM):
                nc.tensor.matmul(wups[:], wu[:], wu[:],
                                 start=(i == 0), stop=(i == NWARM - 1))
            wuo = wp.tile([P, P], FDT, tag="warmo", name="warm_out")
            nc.scalar.copy(wuo[:], wups[:])

            # ---- resident weights / constants (kicked after xt(0)/wg) ----
            wg_sb = wp.tile([P, OC, HC, P], DT8, tag="wg", name="wg_sb")
            wc_sb = wp.tile([P, OC, HC, P], DT16, tag="wc", name="wc_sb")
            ctok_sb = cp.tile([P, n_super, 4], FDT, tag="ctok", name="ctok_sb")
            cbt_sb = cp.tile([P, n_super, 16], FDT, tag="cbt", name="cbt_sb")
            mask_sb = cp.tile([P, P], FDT, tag="dmask", name="mask_sb")
            e4_sb = cp.tile([P, P // 4], DT16, tag="e4", name="e4_sb")
            sy_sb = cp.tile([P, 1], FDT, tag="sy", name="sy_sb")

            def load_consts():
                nc.scalar.dma_start(sy_sb[:], syv[:].partition_broadcast(P))
                nc.scalar.dma_start(ctok_sb[:], ctok2[:])
                nc.scalar.dma_start(cbt_sb[:], cbt2[:])
                nc.scalar.dma_start(mask_sb[:], dmask[:])
                nc.scalar.dma_start(e4_sb[:], e4m[:])

            def load_xt(s):
                xt = xp.tile([P, HC, T_SUPER], DT8, tag="xt")
                if s == 0:
                    # first kick on all three queues: the startup flood of
                    # prefetches shares HBM, so give xt(0) a triple share
                    nc.sync.dma_start(xt[:, 0:3], xT4[:, s, 0:3])
                    nc.scalar.dma_start(xt[:, 3:6], xT4[:, s, 3:6])
                    nc.gpsimd.dma_start(xt[:, 6:], xT4[:, s, 6:])
                else:
                    nc.sync.dma_start(xt[:], xT4[:, s])
                return xt

            def load_xg(s):
                xgt = xgp.tile([P, M * H], DT16, tag="xg")
                nc.gpsimd.dma_start(xgt[:], xg[s * b_super:(s + 1) * b_super])
                return xgt

            def y_proj(s, xt, hook=None):
                """Y = X G in fp8 DoubleRow; quantize back to fp8 with scale sy.

                hook() is emitted after the first oc chunk — used to slot the
                tiny modality-sum matmuls right where their input just landed.
                """
                yT = yp.tile([P, OC, T_SUPER], DT8, tag="yT")
                for oc in range(OC):
                    pt = psp.tile([P, T_SUPER], FDT, tag="proj")
                    for kk in range(HC // 2):
                        nc.tensor.matmul(
                            pt[:], wg_sb[:, oc, 2 * kk:2 * kk + 2, :],
                            xt[:, 2 * kk:2 * kk + 2, :],
                            start=(kk == 0), stop=(kk == HC // 2 - 1),
                            perf_mode=DR)
                    nc.scalar.activation(yT[:, oc, :], pt[:], COPY,
                                         scale=sy_sb[:])
                    if oc == 0 and hook is not None:
                        hook()
                return yT

            def gram_part(s, xt, yT):
                """Block-diag gram -> masked conf-scaled copy -> compacted s4.

                s4[p=(4j'+m), tt, n] = conf * (y_{q} . x_{4j+n}), all on-chip.
                """
                gram_sb = gpl.tile([P, 4, P], FDT, tag="gram")
                s4 = smp.tile([P, 4, 4], FDT, tag="s4", name=f"s4_{s}")
                for tt in range(4):
                    gps = psg.tile([P, P], FDT, tag="gram_ps")
                    tsl = slice(tt * P, (tt + 1) * P)
                    for kk in range(HC // 2):
                        nc.tensor.matmul(
                            gps[:], yT[:, 2 * kk:2 * kk + 2, tsl],
                            xt[:, 2 * kk:2 * kk + 2, tsl],
                            start=(kk == 0), stop=(kk == HC // 2 - 1),
                            perf_mode=DR)
                    # psum->sbuf copy doubles as conf scaling + diagonal mask
                    nc.vector.scalar_tensor_tensor(
                        gram_sb[:, tt, :], gps[:], ctok_sb[:, s, tt:tt + 1],
                        mask_sb[:], op0=MUL, op1=MUL)
                    # compact the 4 surviving entries per row: sum over j-groups
                    nc.vector.reduce_sum(
                        s4[:, tt, :],
                        gram_sb[:, tt, :].rearrange("p (j n) -> p n j", n=4),
                        axis=AX)
                return s4

            def sm_tail(k, s4):
                """Token-layout softmax: logits -> exp -> normalize -> p4 (bf16)."""
                scl = smp.tile([P, 16], FDT, tag="scl", name=f"scl{k}")
                nc.gpsimd.tensor_tensor(
                    scl[:], s4[:].rearrange("p a b -> p (a b)"), cbt_sb[:, k],
                    op=ADD)
                ex = smp.tile([P, 16], FDT, tag="ex", name=f"ex{k}")
                nc.scalar.activation(ex[:], scl[:], EXP)
                z4 = smp.tile([P, M], FDT, tag="z4", name=f"z4_{k}")
                nc.vector.reduce_sum(z4[:], ex[:].rearrange("p (t n) -> p t n", n=4),
                                     axis=AX)
                r4 = smp.tile([P, M], FDT, tag="r4", name=f"r4_{k}")
                nc.vector.reciprocal(r4[:], z4[:])
                p4b = smp.tile([P, 16], DT16, tag="p4b", name=f"p4b{k}")
                with nc.allow_low_precision(reason="softmax probs to bf16"):
                    for tt in range(4):
                        nc.vector.tensor_scalar_mul(
                            p4b[:, 4 * tt:4 * tt + 4], ex[:, 4 * tt:4 * tt + 4],
                            r4[:, tt:tt + 1])
                return p4b

            def e4_mm(k, p4b):
                """Sum softmax probs over the 4 modalities -> w4[j, n] via PE."""
                w4ps = psw.tile([32, 16], FDT, tag="w4ps")
                for tt in range(4):
                    nc.tensor.matmul(w4ps[:, 4 * tt:4 * tt + 4], e4_sb[:],
                                     p4b[:, 4 * tt:4 * tt + 4],
                                     start=True, stop=True)
                w4c = smp.tile([32, 16], FDT, tag="w4c", name=f"w4c{k}")
                nc.scalar.copy(w4c[:], w4ps[:])
                # partition scatter j' -> 32*tt + j' (tiny SB->SB DMAs)
                w4 = smp.tile([P, M], FDT, tag="w4", name=f"w4_{k}")
                for tt in range(4):
                    nc.gpsimd.dma_start(w4[32 * tt:32 * tt + 32, :],
                                        w4c[:, 4 * tt:4 * tt + 4])
                return w4

            def combine(k, w4, xgt, zb2, qt=None):
                """Convex combine on DVE; qt=(zT2, half) kicks quarter transposes."""
                with nc.allow_low_precision(reason="convex combine, fp16 acc"):
                    HH = H // 2
                    halves = ((0, HH), (HH, H)) if qt is not None else ((0, H),)
                    for lo, hi in halves:
                        zb = zb2[:, k % 2, lo:hi]
                        nc.vector.tensor_scalar_mul(zb, xgt[:, lo:hi],
                                                    w4[:, 0:1])
                        for n in range(1, M):
                            nc.vector.scalar_tensor_tensor(
                                zb, xgt[:, n * H + lo:n * H + hi],
                                w4[:, n:n + 1], zb, op0=MUL, op1=ADD)
                        if qt is not None:
                            zT2, half = qt
                            c0 = 8 * half + lo // P
                            c1 = 8 * half + hi // P
                            nc.scalar.dma_start_transpose(
                                zT2[:, c0:c1, :], zb2[:, half, lo:hi])

            def transpose_pair(zb2):
                zT2 = ztp.tile([P, 16, P], DT16, tag="zT2")
                nc.scalar.dma_start_transpose(zT2[:], zb2[:].rearrange("p a b -> p (a b)"))
                # logical row r = pair*1024 + h  ->  partition h%128, chunk pair*8 + h//128
                return zT2

            def transpose_half(zb2, half, zT2=None):
                if zT2 is None:
                    zT2 = ztp.tile([P, 16, P], DT16, tag="zT2")
                nc.scalar.dma_start_transpose(zT2[:, 8 * half:8 * (half + 1), :],
                                            zb2[:, half, :])
                return zT2

            def out_pair(pr, zT2, mid=None):
                zv = zT2[:].rearrange("p (a c) b -> p c a b", a=2)
                o_sb = osb.tile([P, OC, 2 * b_super], DT16, tag="osb")
                for oc in range(OC):
                    if oc == 6 and mid is not None:
                        mid()
                    po = pso.tile([P, 2 * b_super], FDT, tag="outp")
                    for hc in range(HC):
                        nc.tensor.matmul(
                            po[:], wc_sb[:, oc, hc, :], zv[:, hc],
                            start=(hc == 0), stop=(hc == HC - 1))
                    if oc % 2 == 0:
                        nc.scalar.copy(o_sb[:, oc, :], po[:])
                    else:
                        nc.vector.tensor_copy(o_sb[:, oc, :], po[:])
                        # stream finished chunk pairs out while the rest computes
                        nc.sync.dma_start(outT3[:, pr, oc - 1:oc + 1],
                                          o_sb[:, oc - 1:oc + 1])

            # ---- software pipeline ----
            # Kick order: wg chunk 0, then xt(0) (split over all 3 queues),
            # then the rest of wg, the small constants, xt(1); xg/wc stream
            # behind on their queues.
            nc.sync.dma_start(wg_sb[:, 0:2], wg2[:, 0:2])
            xts = {0: load_xt(0)}
            nc.sync.dma_start(wg_sb[:, 2:4], wg2[:, 2:4])
            nc.scalar.dma_start(wg_sb[:, 4:6], wg2[:, 4:6])
            nc.gpsimd.dma_start(wg_sb[:, 6:8], wg2[:, 6:8])
            load_consts()
            xts[1] = load_xt(1)
            xgs = {}
            # wc needed first by out_pair(0) at iteration 3 (its oc 6-7 tail
            # ~19us in): spread the 2MB load across iterations 1..3
            wc_sched = {1: range(0, 3), 2: range(3, 6), 3: range(6, OC)}
            sss = {}
            p4s = {}
            w4s = {}
            zbs = {}
            zts = {}
            zth = None

            def sm_step(k):
                p4s[k] = sm_tail(k, sss.pop(k))

            def w4_step(k):
                w4s[k] = e4_mm(k, p4s.pop(k))

            def combine_step(k, qt=None):
                if k % 2 == 0:
                    zbs[k // 2] = zp.tile([P, 2, H], DT16, tag="zb2",
                                          name=f"zb2_{k // 2}")
                combine(k, w4s.pop(k), xgs.pop(k), zbs[k // 2], qt=qt)
                if k % 2 == 1 and k < n_super - 2:
                    zts[k // 2] = transpose_pair(zbs.pop(k // 2))
                elif k == n_super - 2:
                    return transpose_half(zbs[k // 2], 0)
                return None

            # Per body s: softmax tail of s-1 first (its inputs landed last
            # iteration), then Y(s); the tiny modality-sum matmuls for s-1
            # slot in right after Y(s) so they never head-of-line block the
            # PE; gram(s)'s masked copies queue on the DVE before the
            # combine(s-1) STTs so the PE's psum recycling is never hostage
            # to the combine.
            for s in range(n_super):
                if s >= 1:
                    sm_step(s - 1)
                yT = y_proj(s, xts[s],
                            hook=(lambda: w4_step(s - 1)) if s >= 1 else None)
                if s >= 1:
                    r = combine_step(s - 1)
                    if r is not None:
                        zth = r
                sss[s] = gram_part(s, xts.pop(s), yT)
                for oc in wc_sched.get(s, ()):
                    nc.gpsimd.dma_start(wc_sb[:, oc], wc2[:, oc])
                if s == n_super - 1:
                    # last super: softmax tail emitted in-body so only the
                    # modality-sum + combine + quarter transposes remain
                    sm_step(s)
                if s >= 3 and s % 2 == 1:
                    pr = (s - 3) // 2
                    if s == n_super - 1:
                        # slot the last modality-sum + combine between the
                        # projection chunks of the previous pair
                        def _mid():
                            w4_step(n_super - 1)
                            combine_step(n_super - 1, qt=(zth, 1))
                        out_pair(pr, zts.pop(pr), mid=_mid)
                    else:
                        out_pair(pr, zts.pop(pr))
                if s + 1 < n_super:
                    xgs[s + 1] = load_xg(s + 1)
                if s + 2 < n_super:
                    xts[s + 2] = load_xt(s + 2)
                if s == 0:
                    xgs[0] = load_xg(0)
            # epilogue: final projection on the quarter-transposed last pair
            out_pair(n_pair - 1, zth)
    nc.compile()
    return nc


def _get_nc(n_super=T_CORE // T_SUPER):
    if n_super not in _NC_CACHE:
        _NC_CACHE[n_super] = build_bass(n_super)
    return _NC_CACHE[n_super]


def prep_in_maps(inputs, ncores=NCORES):
    """Host-side: fold weights, pick fp8 scales, prearrange layouts, cast."""
    f32 = np.float32
    f64 = np.float64
    feats = np.asarray(inputs["features"], f32)
    confs = np.asarray(inputs["confidences"], f32).reshape(-1, M)
    Wq = np.asarray(inputs["Wq"], f64)
    Wk = np.asarray(inputs["Wk"], f64)
    Wv = np.asarray(inputs["Wv"], f64)
    Wo = np.asarray(inputs["Wo"], f64)
    bq = np.asarray(inputs["bq"], f64)
    bv = np.asarray(inputs["bv"], f64)
    bo = np.asarray(inputs["bo"], f64)

    s = 1.0 / np.sqrt(H)
    G = (Wq * s).T @ Wk                         # [h, h']
    WcT = ((Wo @ Wv) / 4.0).T                   # [f, o]
    bc_h = (bv @ Wo.T + bo).astype(f32)         # added on host after the run
    w_vec = Wk.T @ (bq * s)                     # beta = X w_vec

    nb = feats.shape[0]
    b_core = nb // ncores
    t_core = b_core * M
    n_super = t_core // T_SUPER
    X = feats.reshape(nb * M, H)

    sx = f32(224.0 / np.abs(X).max())
    sg = f32(224.0 / np.abs(G).max())
    X8 = (X * sx).astype(F8)                    # [t, h]
    wg_f8 = (G * sg).astype(F8)                 # [k, o]
    col_sig = np.sqrt((np.asarray(wg_f8, f32) ** 2).sum(axis=0)).max() * sx
    sy = f32(224.0 / (6.5 * col_sig))           # fp8-Y sigma ~34, 6.5-sigma headroom
    descale = f32(1.0) / (f32(sx) * f32(sx) * f32(sg) * f32(sy))

    # prearranged weights: [p, oc, c, o'] with k = c*128+p, o = oc*128+o'
    wg2_h = np.ascontiguousarray(
        wg_f8.reshape(HC, P, OC, P).transpose(1, 2, 0, 3))
    wc2_h = np.ascontiguousarray(
        WcT.astype(F16).reshape(HC, P, OC, P).transpose(1, 2, 0, 3))

    beta = (X @ w_vec.astype(f32)).astype(f32)  # [nb*M]
    conf_dev = confs * descale
    cbeta = confs[:, :, None] * beta.reshape(nb, M)[:, None, :]   # [b, m, n]
    xg_h = feats.reshape(nb, M * H).astype(F16)

    # constants for the on-chip diagonal extraction
    kk = np.arange(P)
    dmask_h = (kk[None, :] // 4 == kk[:, None] // 4).astype(f32)
    e4m_h = (kk[:, None] // 4 == np.arange(P // 4)[None, :]).astype(F16)

    in_maps = []
    for c in range(ncores):
        tsl = slice(c * t_core, (c + 1) * t_core)
        bsl = slice(c * b_core, (c + 1) * b_core)
        # xT4 [p, s, c, t]: X8[(s t), (c p)] -> transpose
        xT4_h = np.ascontiguousarray(
            X8[tsl].reshape(n_super, T_SUPER, HC, P).transpose(3, 0, 2, 1))
        # ctok [p=4j'+m, s, tt]: conf for token (m) of batch row s*128+tt*32+j'
        ctok2_h = np.ascontiguousarray(
            conf_dev[bsl].reshape(n_super, 4, 32, M)
            .transpose(2, 3, 0, 1).reshape(P, n_super, 4))
        # cbt [p=4j'+m, s, (tt n)]: conf*beta in the same token layout
        cbt2_h = np.ascontiguousarray(
            cbeta[bsl].reshape(n_super, 4, 32, M, 4)
            .transpose(2, 3, 0, 1, 4).reshape(P, n_super, 16))
        in_maps.append({
            "xT4": xT4_h,
            "xg": np.ascontiguousarray(xg_h[bsl]),
            "wg2": wg2_h, "wc2": wc2_h,
            "ctok2": ctok2_h, "cbt2": cbt2_h,
            "dmask": dmask_h, "e4m": e4m_h,
            "syv": np.array([sy], f32),
        })
    return in_maps, bc_h


def install_ntff_hook():
    """Best-effort shim so run_bass_kernel_spmd(trace=True) can profile under axon."""
    import types
    try:
        from antenv.axon_hooks import get_axon_ntff_profile_hook  # noqa: F401
        return True
    except ImportError:
        pass
    try:
        import antenv
        mod = types.ModuleType("antenv.axon_hooks")
        _state = {"hook": None}
        mod.set_axon_ntff_profile_hook = lambda h: _state.__setitem__("hook", h)
        mod.get_axon_ntff_profile_hook = lambda: _state["hook"]
        sys.modules["antenv.axon_hooks"] = mod
        antenv.axon_hooks = mod
        from trn_agent_boot.trn_boot import _ntff_profile_via_ctypes
        hook = _ntff_profile_via_ctypes('/opt/axon/libaxon_pjrt.so')
        if hook is None:
            return False
        mod.set_axon_ntff_profile_hook(hook)
        return True
    except Exception:
        return False


def run(inputs, trace=False, tmpdir=None):
    """Run the 8-core kernel; returns (out [B, H] f32, BassKernelResults)."""
    from concourse.bass_utils import run_bass_kernel_spmd
    nc = _get_nc()
    in_maps, bc_h = prep_in_maps(inputs)
    if trace:
        install_ntff_hook()
    res = run_bass_kernel_spmd(nc, in_maps, core_ids=list(range(NCORES)),
                               trace=trace, tmpdir=tmpdir)
    # outT3 [p, pr, oc, b2]: out[b, o] with b = (2pr + b2//128)*128 + b2%128,
    # o = oc*128 + p
    outs = []
    for o in res.results:
        o3 = np.asarray(o["outT3"], np.float32)     # [128, n_pair, 8, 256]
        npair = o3.shape[1]
        o4 = o3.reshape(P, npair, OC, 2, 128)       # [p, pr, oc, half, 128]
        # -> [pr, half, b(128), oc, p]
        out_c = o4.transpose(1, 3, 4, 2, 0).reshape(npair * 2 * 128, H)
        outs.append(out_c)
    out = np.concatenate(outs, axis=0)
    out += bc_h[None, :]
    return out, res


def kernel(**inputs):
    out, _ = run(inputs, trace=False)
    return out
